# revision 1
# baseline (speedup 1.0000x reference)
"""GCN (3-layer GraphConv, norm='right') Trainium2 Bass kernel.

Strategy: single NeuronCore, single launch. Per layer:
  gather y[src] rows (256B each) from a DRAM table via dma_gather,
  aggregate per 128-dst-node block with one-hot S-matrix matmuls into PSUM
  (inv_deg folded into S), epilogue applies bias/relu and the next layer's
  projection, writing the next gather table.

Edges are grouped by dst block and split into two streams by src half
(A: src<25088, B: src>=25088) because dma_gather indices are int16.
Per-(block,stream) edge lists are padded to multiples of 128; padding
edges carry slot=999 (matches no dst slot) and inv_deg=0, so they
contribute exactly zero.
"""
import numpy as np

import concourse.bass as bass
import concourse.tile as tile
from concourse import bacc, mybir
from concourse.bass_utils import run_bass_kernel_spmd

N_NODES = 50000
N_EDGES = 800000
IN_FEATS, F, N_CLASSES = 128, 64, 40
NBLK = (N_NODES + 127) // 128          # 391
NROWS = NBLK * 128                     # 50048
HSPLIT = 25088                         # rows [0,HSPLIT) -> stream A
TPC = 16                               # tiles per gather chunk (4096 idxs)
ACT_EVERY = 10 ** 9                    # S-builds stay on VectorE (ACT is slower)

_cache = {}


def _pack_stream(srcv, slotv, invdv, blkv, nblk, base):
    """Pad per-block edge groups to multiples of 128 tiles; return arrays."""
    cnt = np.bincount(blkv, minlength=nblk)
    tiles = (cnt + 127) // 128
    T = int(tiles.sum())
    starts = np.concatenate([[0], np.cumsum(cnt)[:-1]])
    tile_starts = np.concatenate([[0], np.cumsum(tiles)[:-1]])
    idx_pad = np.zeros(T * 128, dtype=np.int16)
    slot_pad = np.full(T * 128, 999.0, dtype=np.float32)
    invd_pad = np.zeros(T * 128, dtype=np.float32)
    if len(srcv):
        rank = np.arange(len(srcv)) - np.repeat(starts, cnt)
        pos = np.repeat(tile_starts * 128, cnt) + rank
        idx_pad[pos] = (srcv - base).astype(np.int16)
        slot_pad[pos] = slotv
        invd_pad[pos] = invdv
    # idx dram layout: index i of the stream at [i%16, i//16], replicated x8
    idx_dram = np.tile(idx_pad.reshape(-1, 16).T, (8, 1)).copy()  # [128, T*8]
    slot_t = slot_pad.reshape(T, 128).T.copy()                    # [128, T]
    invd_t = invd_pad.reshape(T, 128).T.copy()
    return idx_dram, slot_t, invd_t, tiles, tile_starts, T


def _meta3(S):
    """Per-chunk-interleaved [slot | invd] array: [128, 2*T]."""
    T = S[5]
    out = np.empty((128, 2 * max(T, 1)), dtype=np.float32)
    for ch in range((T + TPC - 1) // TPC):
        nt = min(TPC, T - ch * TPC)
        base = 2 * ch * TPC
        out[:, base:base + nt] = S[1][:, ch * TPC:ch * TPC + nt]
        out[:, base + nt:base + 2 * nt] = S[2][:, ch * TPC:ch * TPC + nt]
    return np.ascontiguousarray(out)


def _prep(features, src, dst, W0, b0, W1, b1, W2, b2):
    deg = np.bincount(dst, minlength=N_NODES).astype(np.float32)
    invd = (1.0 / np.maximum(deg, 1.0)).astype(np.float32)

    order = np.argsort(dst, kind="stable")
    dst_s = dst[order].astype(np.int64)
    src_s = src[order].astype(np.int64)
    blk = dst_s // 128
    slot = (dst_s % 128).astype(np.float32)
    invd_e = invd[dst_s]

    am = src_s < HSPLIT
    A = _pack_stream(src_s[am], slot[am], invd_e[am], blk[am], NBLK, 0)
    B = _pack_stream(src_s[~am], slot[~am], invd_e[~am], blk[~am], NBLK, HSPLIT)

    xT = np.zeros((IN_FEATS, NROWS), dtype=np.float32)
    xT[:, :N_NODES] = np.ascontiguousarray(features.T)

    W2p = np.zeros((F, F), dtype=np.float32)
    W2p[:, :N_CLASSES] = W2[:, :N_CLASSES]
    b2p = np.zeros((F, 1), dtype=np.float32)
    b2v = np.asarray(b2).reshape(-1)
    b2p[:min(len(b2v), F), 0] = b2v[:min(len(b2v), F)]

    in_map = {
        "xT": xT,
        "W0": np.ascontiguousarray(W0.astype(np.float32)),
        "W1": np.ascontiguousarray(W1.astype(np.float32)),
        "W2p": W2p,
        "b0": np.asarray(b0, dtype=np.float32).reshape(F, 1),
        "b1": np.asarray(b1, dtype=np.float32).reshape(F, 1),
        "b2p": b2p,
        "iota": np.tile(np.arange(128, dtype=np.float32), (128, 1)),
        "ident": np.eye(128, dtype=np.float32),
        "idxA": A[0], "metaA": _meta3(A),
        "idxB": B[0], "metaB": _meta3(B),
    }
    sched = {"A": (A[3], A[4], A[5]), "B": (B[3], B[4], B[5])}
    return in_map, sched


def _build(sched):
    TA = sched["A"][2]
    TB = sched["B"][2]

    nc = bacc.Bacc("TRN2", num_devices=1, dynamic_dma_scratch_size=65536)
    dt = mybir.dt.float32

    xT_in = nc.dram_tensor("xT", [IN_FEATS, NROWS], dt, kind="ExternalInput")
    W0_in = nc.dram_tensor("W0", [IN_FEATS, F], dt, kind="ExternalInput")
    W1_in = nc.dram_tensor("W1", [F, F], dt, kind="ExternalInput")
    W2_in = nc.dram_tensor("W2p", [F, F], dt, kind="ExternalInput")
    b0_in = nc.dram_tensor("b0", [F, 1], dt, kind="ExternalInput")
    b1_in = nc.dram_tensor("b1", [F, 1], dt, kind="ExternalInput")
    b2_in = nc.dram_tensor("b2p", [F, 1], dt, kind="ExternalInput")
    iota_in = nc.dram_tensor("iota", [128, 128], dt, kind="ExternalInput")
    ident_in = nc.dram_tensor("ident", [128, 128], dt, kind="ExternalInput")
    meta_in = {}
    for s, T in (("A", TA), ("B", TB)):
        meta_in["idx" + s] = nc.dram_tensor("idx" + s, [128, max(T, 1) * 8], mybir.dt.int16, kind="ExternalInput")
        meta_in["meta" + s] = nc.dram_tensor("meta" + s, [128, max(T, 1) * 2], dt, kind="ExternalInput")
    out = nc.dram_tensor("out", [NROWS, F], dt, kind="ExternalOutput")

    with tile.TileContext(nc) as tc:
        with tc.tile_pool(name="const", bufs=1) as cp, \
             tc.tile_pool(name="dram", bufs=1, space="DRAM") as dram, \
             tc.tile_pool(name="msg", bufs=5) as mp, \
             tc.tile_pool(name="midx", bufs=4) as ip, \
             tc.tile_pool(name="marr", bufs=4) as ap_, \
             tc.tile_pool(name="stl", bufs=12) as sp, \
             tc.tile_pool(name="xblk", bufs=4) as xp, \
             tc.tile_pool(name="ep", bufs=4) as epp, \
             tc.tile_pool(name="agg", bufs=3, space="PSUM") as pp, \
             tc.tile_pool(name="eps", bufs=2, space="PSUM") as pp2:

            iota_t = cp.tile([128, 128], dt)
            nc.sync.dma_start(iota_t[:], iota_in[:])
            ident_t = cp.tile([128, 128], dt)
            nc.sync.dma_start(ident_t[:], ident_in[:])
            W0_t = cp.tile([IN_FEATS, F], dt)
            nc.sync.dma_start(W0_t[:], W0_in[:])
            W1_t = cp.tile([F, F], dt)
            nc.sync.dma_start(W1_t[:], W1_in[:])
            W2_t = cp.tile([F, F], dt)
            nc.sync.dma_start(W2_t[:], W2_in[:])
            b0_t = cp.tile([F, 1], dt)
            nc.sync.dma_start(b0_t[:], b0_in[:])
            b1_t = cp.tile([F, 1], dt)
            nc.sync.dma_start(b1_t[:], b1_in[:])
            b2_t = cp.tile([F, 1], dt)
            nc.sync.dma_start(b2_t[:], b2_in[:])

            tbl = []
            for l in range(3):
                tb = dram.tile([NROWS, F], dt, tag=f"t{l}")
                tbl.append(tb)

            # ---- Layer-1 projection: t0 = X @ W0 ----
            for b in range(NBLK):
                xb = xp.tile([IN_FEATS, 128], dt, tag="xb")
                nc.sync.dma_start(xb[:], xT_in[:, b * 128:(b + 1) * 128])
                yp = pp2.tile([128, F], dt, tag="pj")
                nc.tensor.matmul(yp[:], xb[:], W0_t[:], start=True, stop=True)
                ys = epp.tile([128, F], dt, tag="ysb")
                nc.vector.tensor_copy(ys[:], yp[:])
                nc.sync.dma_start(tbl[0][b * 128:(b + 1) * 128, :], ys[:])

            # ---- Layers ----
            tile_ctr = 0
            for l in range(3):
                table = tbl[l]
                views = {"A": table[0:HSPLIT, :], "B": table[HSPLIT:NROWS, :]}
                msgs = {}
                arrs = {}
                for s in ("A", "B"):
                    T = sched[s][2]
                    n_chunks = (T + TPC - 1) // TPC
                    msgs[s] = []
                    arrs[s] = []
                    SC = 4  # chunks per idx/meta load (bigger DMA descriptors)
                    idx_sc = m3_sc = None
                    for ch in range(n_chunks):
                        nt = min(TPC, T - ch * TPC)
                        k = ch % SC
                        if k == 0:
                            ntot = min(SC * TPC, T - ch * TPC)
                            idx_sc = ip.tile([128, ntot * 8], mybir.dt.int16, tag="idx" + s)
                            nc.sync.dma_start(idx_sc[:], meta_in["idx" + s][:, ch * TPC * 8: ch * TPC * 8 + ntot * 8])
                            m3_sc = ap_.tile([128, 2 * ntot], dt, tag="m3" + s)
                            nc.sync.dma_start(m3_sc[:], meta_in["meta" + s][:, 2 * ch * TPC: 2 * ch * TPC + 2 * ntot])
                        idx_t = idx_sc[:, k * TPC * 8: k * TPC * 8 + nt * 8]
                        base = 2 * k * TPC
                        sl = m3_sc[:, base:base + nt]
                        iv = m3_sc[:, base + nt:base + 2 * nt]
                        msg = mp.tile([128, nt, F], dt, tag="msg" + s)
                        nc.gpsimd.dma_gather(
                            msg[:], views[s], idx_t,
                            num_idxs=nt * 128, num_idxs_reg=nt * 128,
                            elem_size=F, single_packet=False)
                        msgs[s].append(msg)
                        arrs[s].append((sl, iv))

                for b in range(NBLK):
                    refs = []
                    for s in ("A", "B"):
                        tiles, tstarts, _T = sched[s]
                        for t in range(int(tstarts[b]), int(tstarts[b] + tiles[b])):
                            refs.append((s, t // TPC, t % TPC))
                    agg = pp.tile([128, F], dt, tag="agg")
                    nt_b = len(refs)
                    for i, (s, ch, col) in enumerate(refs):
                        sl, iv = arrs[s][ch]
                        S = sp.tile([128, 128], dt, tag="S")
                        tile_ctr += 1
                        nc.vector.tensor_scalar(
                            S[:], iota_t[:], sl[:, col:col + 1], iv[:, col:col + 1],
                            mybir.AluOpType.is_equal, mybir.AluOpType.mult)
                        nc.tensor.matmul(agg[:], S[:], msgs[s][ch][:, col, :],
                                         start=(i == 0), stop=(i == nt_b - 1))

                    # epilogue
                    t0 = epp.tile([128, F], dt, tag="t0")
                    if nt_b == 0:
                        nc.vector.memset(t0[:], 0.0)
                    else:
                        nc.vector.tensor_copy(t0[:], agg[:])
                    t0T = pp2.tile([F, 128], dt, tag="t0T")
                    nc.tensor.transpose(t0T[:], t0[:], ident_t[:])
                    rows = slice(b * 128, (b + 1) * 128)
                    if l == 0:
                        hT = epp.tile([F, 128], dt, tag="hT")
                        nc.scalar.activation(hT[:], t0T[:], mybir.ActivationFunctionType.Relu,
                                             bias=b0_t[:, 0:1], scale=1.0)
                        yT = pp2.tile([F, 128], dt, tag="pj")
                        nc.tensor.matmul(yT[:], W1_t[:], hT[:], start=True, stop=True)
                        yTs = epp.tile([F, 128], dt, tag="yTs")
                        nc.vector.tensor_copy(yTs[:], yT[:])
                        yps = pp2.tile([128, F], dt, tag="pj")
                        nc.tensor.transpose(yps[:], yTs[:], ident_t[0:F, 0:F])
                        ysb = epp.tile([128, F], dt, tag="ysb")
                        nc.vector.tensor_copy(ysb[:], yps[:])
                        nc.sync.dma_start(tbl[1][rows, :], ysb[:])
                    elif l == 1:
                        hT = epp.tile([F, 128], dt, tag="hT")
                        nc.scalar.activation(hT[:], t0T[:], mybir.ActivationFunctionType.Relu,
                                             bias=b1_t[:, 0:1], scale=1.0)
                        hps = pp2.tile([128, F], dt, tag="pj")
                        nc.tensor.transpose(hps[:], hT[:], ident_t[0:F, 0:F])
                        hsb = epp.tile([128, F], dt, tag="ysb")
                        nc.vector.tensor_copy(hsb[:], hps[:])
                        nc.sync.dma_start(tbl[2][rows, :], hsb[:])
                    else:
                        # out = aggT.T @ W2p + b2: project the (normalized) agg
                        aT = epp.tile([F, 128], dt, tag="hT")
                        nc.vector.tensor_copy(aT[:], t0T[:])
                        oT = pp2.tile([F, 128], dt, tag="pj")
                        nc.tensor.matmul(oT[:], W2_t[:], aT[:], start=True, stop=True)
                        oTb = epp.tile([F, 128], dt, tag="yTs")
                        nc.scalar.activation(oTb[:], oT[:], mybir.ActivationFunctionType.Identity,
                                             bias=b2_t[:, 0:1], scale=1.0)
                        ops_ = pp2.tile([128, F], dt, tag="pj")
                        nc.tensor.transpose(ops_[:], oTb[:], ident_t[0:F, 0:F])
                        osb = epp.tile([128, F], dt, tag="ysb")
                        nc.vector.tensor_copy(osb[:], ops_[:])
                        nc.sync.dma_start(out[rows, :], osb[:])

    nc.compile()
    return nc


def kernel(features, src, dst, W0, b0, W1, b1, W2, b2):
    features = np.asarray(features, dtype=np.float32)
    src = np.asarray(src).astype(np.int64)
    dst = np.asarray(dst).astype(np.int64)
    in_map, sched = _prep(features, src, dst,
                          np.asarray(W0), np.asarray(b0), np.asarray(W1),
                          np.asarray(b1), np.asarray(W2), np.asarray(b2))
    key = (sched["A"][2], sched["B"][2],
           tuple(sched["A"][0].tolist()), tuple(sched["B"][0].tolist()))
    if _cache.get("key") != key:
        _cache["nc"] = _build(sched)
        _cache["key"] = key
    nc = _cache["nc"]
    res = run_bass_kernel_spmd(nc, [in_map], core_ids=[0])
    full = res.results[0]["out"]
    return np.ascontiguousarray(full[:N_NODES, :N_CLASSES])



# revision 8
# speedup vs baseline: 5.1047x; 5.1047x over previous
"""GCN (3-layer GraphConv, norm='right') — 8-core SPMD Trainium2 Bass kernel.

Strategy (src-sharded edges + per-layer ReduceScatter):
  Nodes are split into 8 contiguous shards of 49 blocks (6272 rows). Core c
  owns edges whose SRC lies in its shard, so every gather reads only the
  core-local projected table (single int16 index stream, 256B bf16 rows).
  Per layer, each core aggregates its edges' messages into a FULL 392-block
  partial table (one-hot S-matmuls with inv_deg folded in, bf16), then one
  ReduceScatter(add) sums partials and hands each core its dst shard. The
  epilogue applies bias/relu and the next layer's projection locally — no
  AllGather is ever needed because next-layer gathers only read local rows.

Layer l table = h_{l-1} @ W_l (pre-projected, fp16, 128-wide rows so each
gather is one 256B granule; cols 64:128 are junk and never read). fp16 (not
bf16) because 8 independently-rounded partials + the RS reduce compound the
quantization error ~10x; fp16's 2^-11 keeps the final rel-err ~2e-3.
"""
import numpy as np
import ml_dtypes

import concourse.bass as bass
import concourse.tile as tile
from concourse import bacc, mybir
from concourse.bass_utils import run_bass_kernel_spmd

BF = np.float16
N_NODES = 50000
N_EDGES = 800000
F_IN, F, N_CLASSES = 128, 64, 40
NCORES = 8
BLKS = 392                     # global dst blocks (50176 rows)
NROWS = BLKS * 128
SHARD_BLKS = BLKS // NCORES    # 49
SHARD = SHARD_BLKS * 128       # 6272
TPC = 32                       # tiles per gather chunk
PGRP = 56                      # partial-write staging group (392 = 7*56)
TGRP = 7                       # table-write staging group (49 = 7*7)
BAD = 999.0
DVE_OF_10 = 7                  # S-builds: 7/10 on DVE, 3/10 on gpsimd(Pool)

_cache = {}


def _prep(features, src, dst, W0, b0, W1, b1, W2, b2):
    deg = np.bincount(dst, minlength=NROWS).astype(np.float32)
    invd = (1.0 / np.maximum(deg, 1.0)).astype(np.float32)

    src = src.astype(np.int64)
    dst = dst.astype(np.int64)
    core = src // SHARD
    order = np.lexsort((dst, core))
    src_s, dst_s, core_s = src[order], dst[order], core[order]
    blk = dst_s >> 7
    slot = (dst_s & 127).astype(np.float32)
    iv_e = invd[dst_s]
    loc = src_s - core_s * SHARD

    cnt = np.zeros((NCORES, BLKS), np.int64)
    np.add.at(cnt, (core_s, blk), 1)
    n_b = cnt.max(axis=0)
    S_off = np.concatenate([[0], np.cumsum(n_b)[:-1]])
    NS = int(n_b.sum())
    T = (NS + 127) // 128
    NSP = T * 128

    idx_flat = np.zeros((NCORES, NSP), np.int16)
    sl_flat = np.full((NCORES, NSP), BAD, np.float32)
    iv_flat = np.zeros((NCORES, NSP), np.float32)
    grp = core_s * BLKS + blk
    grp_cnt = cnt.reshape(-1)
    grp_starts = np.concatenate([[0], np.cumsum(grp_cnt)[:-1]])
    rank = np.arange(len(src_s)) - grp_starts[grp]
    pos = S_off[blk] + rank
    idx_flat[core_s, pos] = loc.astype(np.int16)
    sl_flat[core_s, pos] = slot
    iv_flat[core_s, pos] = iv_e

    # shared op schedule: one S-matmul per (tile, block) overlap
    ops = []
    blk_first = np.zeros(BLKS, np.int64)
    blk_nops = np.zeros(BLKS, np.int64)
    for b in range(BLKS):
        if n_b[b] == 0:
            continue
        t0 = int(S_off[b]) // 128
        t1 = int(S_off[b] + n_b[b] - 1) // 128
        blk_first[b] = len(ops)
        blk_nops[b] = t1 - t0 + 1
        for t in range(t0, t1 + 1):
            ops.append((t, b))
    NOPS = len(ops)

    sl_cols = np.full((NCORES, 128, NOPS), BAD, np.float32)
    iv_cols = np.zeros((NCORES, 128, NOPS), np.float32)
    for o, (t, b) in enumerate(ops):
        s0 = t * 128
        lo = max(int(S_off[b]), s0)
        hi = min(int(S_off[b] + n_b[b]), s0 + 128)
        sl_cols[:, lo - s0:hi - s0, o] = sl_flat[:, lo:hi]
        iv_cols[:, lo - s0:hi - s0, o] = iv_flat[:, lo:hi]

    idxd = np.stack([np.tile(idx_flat[c].reshape(-1, 16).T, (8, 1))
                     for c in range(NCORES)])          # [NCORES, 128, T*8]

    xTp = np.zeros((F_IN, NCORES * SHARD), dtype=BF)
    xTp[:, :N_NODES] = np.ascontiguousarray(features.T).astype(BF)

    W2p = np.zeros((F, F), np.float32)
    W2p[:, :N_CLASSES] = np.asarray(W2, np.float32)[:, :N_CLASSES]
    b2v = np.asarray(b2, np.float32).reshape(-1)
    b2p = np.zeros((F,), np.float32)
    b2p[:min(len(b2v), F)] = b2v[:min(len(b2v), F)]

    in_maps = []
    for c in range(NCORES):
        in_maps.append({
            "xT": np.ascontiguousarray(xTp[:, c * SHARD:(c + 1) * SHARD]),
            "idx": np.ascontiguousarray(idxd[c]),
            "sl": np.ascontiguousarray(sl_cols[c]),
            "iv": np.ascontiguousarray(iv_cols[c]),
            "W0b": np.asarray(W0, np.float32).astype(BF),
            "W1b": np.asarray(W1, np.float32).astype(BF),
            "W2b": W2p.astype(BF),
            "b0": np.asarray(b0, np.float32).reshape(F, 1),
            "b1": np.asarray(b1, np.float32).reshape(F, 1),
            "b2bc": np.tile(b2p, (128, 1)),
            "iota": np.tile(np.arange(128, dtype=np.float32),
                            (128, 1)).astype(BF),
            "ident": np.eye(128, dtype=np.float32).astype(BF),
        })
    sched = {"T": T, "NOPS": NOPS, "ops": ops,
             "blk_first": blk_first, "blk_nops": blk_nops}
    return in_maps, sched


def _build(sched):
    T, NOPS = sched["T"], sched["NOPS"]
    ops = sched["ops"]
    blk_first, blk_nops = sched["blk_first"], sched["blk_nops"]

    nc = bacc.Bacc("TRN2", num_devices=NCORES,
                   dynamic_dma_scratch_size=65536)
    dt = mybir.dt
    f32, bf16, i16 = dt.float32, dt.float16, dt.int16

    xT_in = nc.dram_tensor("xT", [F_IN, SHARD], bf16, kind="ExternalInput")
    idx_in = nc.dram_tensor("idx", [128, T * 8], i16, kind="ExternalInput")
    sl_in = nc.dram_tensor("sl", [128, NOPS], f32, kind="ExternalInput")
    iv_in = nc.dram_tensor("iv", [128, NOPS], f32, kind="ExternalInput")
    W0_in = nc.dram_tensor("W0b", [F_IN, F], bf16, kind="ExternalInput")
    W1_in = nc.dram_tensor("W1b", [F, F], bf16, kind="ExternalInput")
    W2_in = nc.dram_tensor("W2b", [F, F], bf16, kind="ExternalInput")
    b0_in = nc.dram_tensor("b0", [F, 1], f32, kind="ExternalInput")
    b1_in = nc.dram_tensor("b1", [F, 1], f32, kind="ExternalInput")
    b2_in = nc.dram_tensor("b2bc", [128, F], f32, kind="ExternalInput")
    iota_in = nc.dram_tensor("iota", [128, 128], bf16, kind="ExternalInput")
    ident_in = nc.dram_tensor("ident", [128, 128], bf16, kind="ExternalInput")
    out = nc.dram_tensor("out", [SHARD, F], f32, kind="ExternalOutput")

    n_chunks = (T + TPC - 1) // TPC

    with tile.TileContext(nc) as tc:
        with tc.tile_pool(name="const", bufs=1) as cp, \
             tc.tile_pool(name="dram", bufs=1, space="DRAM") as dram, \
             tc.tile_pool(name="msg", bufs=3) as mp, \
             tc.tile_pool(name="stl", bufs=10) as spl, \
             tc.tile_pool(name="xblk", bufs=3) as xp, \
             tc.tile_pool(name="pstg", bufs=2) as pstg, \
             tc.tile_pool(name="tstg", bufs=2) as tstg, \
             tc.tile_pool(name="epi", bufs=4) as ep, \
             tc.tile_pool(name="aggp", bufs=4, space="PSUM") as pp, \
             tc.tile_pool(name="trp", bufs=2, space="PSUM") as pt, \
             tc.tile_pool(name="prp", bufs=2, space="PSUM") as pp2:

            iota_t = cp.tile([128, 128], bf16)
            nc.sync.dma_start(iota_t[:], iota_in[:])
            ident_t = cp.tile([128, 128], bf16)
            nc.sync.dma_start(ident_t[:], ident_in[:])
            W0_t = cp.tile([F_IN, F], bf16)
            nc.sync.dma_start(W0_t[:], W0_in[:])
            W1_t = cp.tile([F, F], bf16)
            nc.sync.dma_start(W1_t[:], W1_in[:])
            W2_t = cp.tile([F, F], bf16)
            nc.sync.dma_start(W2_t[:], W2_in[:])
            b0_t = cp.tile([F, 1], f32)
            nc.sync.dma_start(b0_t[:], b0_in[:])
            b1_t = cp.tile([F, 1], f32)
            nc.sync.dma_start(b1_t[:], b1_in[:])
            b2_t = cp.tile([128, F], f32)
            nc.sync.dma_start(b2_t[:], b2_in[:])
            idx_sb = cp.tile([128, T * 8], i16)
            nc.sync.dma_start(idx_sb[:], idx_in[:])
            sl_sb = cp.tile([128, NOPS], f32)
            nc.sync.dma_start(sl_sb[:], sl_in[:])
            iv_sb = cp.tile([128, NOPS], f32)
            nc.sync.dma_start(iv_sb[:], iv_in[:])

            tbls = [dram.tile([SHARD, 128], bf16, tag=f"tbl{l}",
                              name=f"tbl{l}") for l in range(3)]
            parts = [dram.tile([NROWS, F], bf16, tag=f"part{l}",
                               name=f"part{l}") for l in range(3)]
            aggs_d = [dram.tile([SHARD, F], bf16, tag=f"agg{l}",
                                name=f"agg{l}") for l in range(3)]

            # ---- Phase A: tbl0 = X @ W0 (local shard) ----
            stg = None
            for j in range(SHARD_BLKS):
                xb = xp.tile([F_IN, 128], bf16, tag="xb")
                nc.sync.dma_start(xb[:], xT_in[:, j * 128:(j + 1) * 128])
                yp = pp2.tile([128, F], f32, tag="prj")
                nc.tensor.matmul(yp[:], xb[:], W0_t[:], start=True, stop=True)
                gi = j % TGRP
                if gi == 0:
                    stg = tstg.tile([128, TGRP, 128], bf16, tag="tstg")
                nc.scalar.activation(stg[:, gi, 0:F], yp[:],
                                     mybir.ActivationFunctionType.Identity,
                                     bias=0.0, scale=1.0)
                if gi == TGRP - 1:
                    g0 = j - gi
                    dst_ap = tbls[0][g0 * 128:(g0 + TGRP) * 128, :] \
                        .rearrange("(g p) c -> p g c", p=128)
                    nc.sync.dma_start(dst_ap, stg[:])

            # ---- Layers ----
            for l in range(3):
                tbl = tbls[l]
                # gather chunks are issued lazily as the op walk reaches them
                msgs = {}

                def ensure_chunk(ch):
                    if ch in msgs:
                        return
                    nt = min(TPC, T - ch * TPC)
                    msg = mp.tile([128, nt, 128], bf16, tag="msg")
                    nc.gpsimd.dma_gather(
                        msg[:], tbl[:],
                        idx_sb[:, ch * TPC * 8: ch * TPC * 8 + nt * 8],
                        num_idxs=nt * 128, num_idxs_reg=nt * 128,
                        elem_size=128, single_packet=False)
                    msgs[ch] = msg

                pstg_t = None
                for b in range(BLKS):
                    gi = b % PGRP
                    if gi == 0:
                        pstg_t = pstg.tile([128, PGRP, F], bf16, tag="pstg")
                    if blk_nops[b] == 0:
                        nc.vector.memset(pstg_t[:, gi, :], 0.0)
                    else:
                        o0 = int(blk_first[b])
                        nops = int(blk_nops[b])
                        ps = pp.tile([128, F], f32, tag="agg")
                        for k in range(nops):
                            o = o0 + k
                            t, _b = ops[o]
                            ch = t // TPC
                            ensure_chunk(ch)
                            if ch + 1 < n_chunks and t % TPC >= TPC - 2:
                                ensure_chunk(ch + 1)
                            S = spl.tile([128, 128], bf16, tag="S")
                            eng = nc.vector if (o % 10) < DVE_OF_10 \
                                else nc.gpsimd
                            eng.tensor_scalar(
                                S[:], iota_t[:], sl_sb[:, o:o + 1],
                                iv_sb[:, o:o + 1],
                                mybir.AluOpType.is_equal,
                                mybir.AluOpType.mult)
                            nc.tensor.matmul(
                                ps[:], S[:],
                                msgs[ch][:, t % TPC, 0:F],
                                start=(k == 0), stop=(k == nops - 1))
                        nc.scalar.activation(
                            pstg_t[:, gi, :], ps[:],
                            mybir.ActivationFunctionType.Identity,
                            bias=0.0, scale=1.0)
                    if gi == PGRP - 1:
                        g0 = b - gi
                        dst_ap = parts[l][g0 * 128:(g0 + PGRP) * 128, :] \
                            .rearrange("(g p) c -> p g c", p=128)
                        nc.sync.dma_start(dst_ap, pstg_t[:])

                nc.gpsimd.collective_compute(
                    "ReduceScatter", mybir.AluOpType.add,
                    replica_groups=[list(range(NCORES))],
                    ins=[parts[l][:].opt()], outs=[aggs_d[l][:].opt()])

                # ---- epilogue on my shard ----
                agg_sb = ep.tile([128, SHARD_BLKS, F], bf16, tag="aggsb")
                nc.sync.dma_start(
                    agg_sb[:],
                    aggs_d[l][:].rearrange("(g p) c -> p g c", p=128))
                if l < 2:
                    W_next = W1_t if l == 0 else W2_t
                    b_cur = b0_t if l == 0 else b1_t
                    stg2 = None
                    for i in range(SHARD_BLKS):
                        hp = pt.tile([F, 128], bf16, tag="hT")
                        nc.tensor.transpose(hp[:], agg_sb[:, i, :],
                                            ident_t[:])
                        hT = ep.tile([F, 128], bf16, tag="hTs")
                        nc.scalar.activation(
                            hT[:], hp[:],
                            mybir.ActivationFunctionType.Relu,
                            bias=b_cur[:, 0:1], scale=1.0)
                        tp = pp2.tile([128, F], f32, tag="prj")
                        nc.tensor.matmul(tp[:], hT[:], W_next[:],
                                         start=True, stop=True)
                        gi = i % TGRP
                        if gi == 0:
                            stg2 = tstg.tile([128, TGRP, 128], bf16,
                                             tag="tstg")
                        nc.scalar.activation(
                            stg2[:, gi, 0:F], tp[:],
                            mybir.ActivationFunctionType.Identity,
                            bias=0.0, scale=1.0)
                        if gi == TGRP - 1:
                            g0 = i - gi
                            dst_ap = tbls[l + 1][g0 * 128:(g0 + TGRP) * 128, :] \
                                .rearrange("(g p) c -> p g c", p=128)
                            nc.sync.dma_start(dst_ap, stg2[:])
                else:
                    ostg = ep.tile([128, SHARD_BLKS, F], f32, tag="ostg")
                    for i in range(SHARD_BLKS):
                        tmp = ep.tile([128, F], f32, tag="otmp")
                        nc.vector.tensor_copy(tmp[:], agg_sb[:, i, :])
                        nc.vector.tensor_tensor(
                            ostg[:, i, :], tmp[:], b2_t[:],
                            mybir.AluOpType.add)
                    nc.sync.dma_start(
                        out[:].rearrange("(g p) c -> p g c", p=128),
                        ostg[:])

    nc.compile()
    return nc


def kernel(features, src, dst, W0, b0, W1, b1, W2, b2):
    features = np.asarray(features, dtype=np.float32)
    src = np.asarray(src).astype(np.int64)
    dst = np.asarray(dst).astype(np.int64)
    in_maps, sched = _prep(features, src, dst,
                           np.asarray(W0), np.asarray(b0), np.asarray(W1),
                           np.asarray(b1), np.asarray(W2), np.asarray(b2))
    key = (sched["T"], sched["NOPS"],
           hash(sched["blk_first"].tobytes()),
           hash(sched["blk_nops"].tobytes()))
    if _cache.get("key") != key:
        _cache["nc"] = _build(sched)
        _cache["key"] = key
    nc = _cache["nc"]
    res = run_bass_kernel_spmd(nc, in_maps, core_ids=list(range(NCORES)))
    full = np.concatenate([np.asarray(res.results[c]["out"])
                           for c in range(NCORES)], axis=0)
    return np.ascontiguousarray(full[:N_NODES, :N_CLASSES])
